# revision 13
# baseline (speedup 1.0000x reference)
"""Adaptive max-pool [32,224,224,128] -> [32,7,7,128] on 8 TRN2 NeuronCores.

Data-parallel over batch: each core pools its own [4,224,224,128] slab.
224 = 7 bins x 32, so this is a 32x32/stride-32 max pool.

Per-core plan (memory-bound, ~103 MB HBM read per core). All compute on DVE
(vector engine) so cross-engine sync is minimal:

  Stage A: tiles [128 rows=(b,h), wchunk*32*128c] with 32KB-contiguous
           DMA descriptors; reduce_max over w (innermost via permuted AP)
           -> S1[row, j, c]   (j = w-bin).
  Stage B: DVE 32x32 block transpose of S1 [128, 896]: free block B=(j,cb),
           S2[32a+i, 32B+k] = S1[32a+k, 32B+i]  (a = h-bin in tile, k = row
           in bin, cb = c//32, i = c%32).
  Stage C: reduce_max over k (innermost 32) -> S3[32a+i, rt*32 + (j*4+cb)].
  Stage D: one more block transpose puts (j,cb) into partitions and (rt,i)
           into the free axis -> S4[32a + j*4+cb, rt*32+i]; 4 strided DMAs
           (one per a) scatter to out[(4rt+a), j, 32cb+i].
"""

import sys

if "/opt/trn_rl_repo" not in sys.path:
    sys.path.insert(0, "/opt/trn_rl_repo")

import numpy as np

import concourse.bacc as bacc
import concourse.bass as bass
import concourse.tile as tile
from concourse import mybir
from concourse.bass_utils import run_bass_kernel_spmd

N_CORES = 8
B_FULL, H, W, C = 32, 224, 224, 128
B = B_FULL // N_CORES  # 4 per core
BINS = 7
BIN = 32
ROWS = B * H  # 896 (b,h) rows per core
P = 128
N_ROW_TILES = ROWS // P  # 7
GB_PER_TILE = P // BIN  # 4 h-bins inside each 128-row tile

# w chunking for stage A: one bin (32 w) = 16KB/partition = 2MB per DMA.
# Uniform fine granularity keeps the DVE<->DMA pipeline smooth at the head,
# at ring-completion skews, and at the tail.
W_CHUNKS = [(j, 1) for j in range(BINS)]  # (start bin, n bins)


def build_kernel() -> bass.Bass:
    nc = bacc.Bacc()
    f32 = mybir.dt.float32

    inp = nc.declare_dram_parameter("inputs", [B, H, W, C], f32, isOutput=False)
    out = nc.declare_dram_parameter("out", [B, BINS, BINS, C], f32, isOutput=True)

    in_flat = inp[:, :, :, :].rearrange("b h w c -> (b h) (w c)")  # [896, 28672]
    out_1d = out[:, :, :, :].rearrange("b i j c -> (b i j c)")  # [25088]

    with tile.TileContext(nc) as tc:
        with (
            tc.tile_pool(name="in_pool", bufs=8) as in_pool,
            tc.tile_pool(name="s1_pool", bufs=2) as s1_pool,
            tc.tile_pool(name="s2_pool", bufs=2) as s2_pool,
            tc.tile_pool(name="singles", bufs=1) as singles,
        ):
            s3 = singles.tile([P, N_ROW_TILES, BIN], f32)  # [p', rt, jcb(pad 32)]
            nc.vector.memset(s3, 0.0)
            s4 = singles.tile([P, N_ROW_TILES, BIN], f32)  # [q, rt, i]

            n_dma = 0
            for rt in range(N_ROW_TILES):
                s1 = s1_pool.tile([P, BINS, C], f32, tag="s1")  # [row, j, c]
                r0 = rt * P
                # ---- Stage A: load + contiguous pairwise max tree over w ----
                for j0, nj in W_CHUNKS:
                    t = in_pool.tile([P, nj, BIN, C], f32, tag="in")
                    # alternate the two HWDGE rings (SP / ACT) so DMA fixed
                    # costs overlap
                    eng = nc.sync if n_dma % 2 == 0 else nc.scalar
                    n_dma += 1
                    eng.dma_start(
                        out=t,
                        in_=in_flat[r0 : r0 + P, j0 * BIN * C : (j0 + nj) * BIN * C],
                    )
                    # strided reduce runs at ~0.62 elem/cycle on DVE; a
                    # contiguous in-place halving tree runs at ~1/cycle
                    w = BIN
                    while w > 2:
                        h = w // 2
                        nc.vector.tensor_max(
                            t[:, :, 0:h, :], t[:, :, 0:h, :], t[:, :, h:w, :]
                        )
                        w = h
                    nc.vector.tensor_max(
                        s1[:, j0 : j0 + nj, :], t[:, :, 0, :], t[:, :, 1, :]
                    )
                # ---- Stage B: 32x32 block transpose ----
                s2 = s2_pool.tile([P, BINS * C], f32, tag="s2")
                nc.vector.transpose(s2, s1.rearrange("p j c -> p (j c)"))
                # ---- Stage C: reduce over rows-in-bin (k) ----
                nc.vector.reduce_max(
                    out=s3[:, rt, 0 : BINS * GB_PER_TILE],
                    in_=s2.rearrange("p (bb k) -> p bb k", k=BIN),
                    axis=mybir.AxisListType.X,
                )
                # ---- Stage D (per rt): block transpose of this rt column ----
                nc.vector.transpose(s4[:, rt, :], s3[:, rt, :])
                # ---- output DMAs for this rt (one per h-bin slot a) ----
                # out flat offset = (4*rt + a)*896 + j*128 + 32*cb + i
                for a in range(GB_PER_TILE):
                    dst = bass.AP(
                        tensor=out_1d.tensor,
                        offset=(GB_PER_TILE * rt + a) * BINS * C,
                        ap=[
                            [C, BINS],  # j
                            [BIN, GB_PER_TILE],  # cb
                            [1, BIN],  # i
                        ],
                    )
                    nc.sync.dma_start(
                        out=dst,
                        in_=s4[a * BIN : a * BIN + BINS * GB_PER_TILE, rt, :],
                    )

    nc.compile()
    return nc


_NC_CACHE = None


def kernel(inputs: np.ndarray) -> np.ndarray:
    global _NC_CACHE
    inputs = np.ascontiguousarray(inputs, dtype=np.float32)
    assert inputs.shape == (B_FULL, H, W, C)

    if _NC_CACHE is None:
        _NC_CACHE = build_kernel()
    nc = _NC_CACHE

    shards = np.split(inputs, N_CORES, axis=0)
    in_maps = [{"inputs": s} for s in shards]
    res = run_bass_kernel_spmd(nc, in_maps, core_ids=list(range(N_CORES)))
    outs = [res.results[i]["out"] for i in range(N_CORES)]
    return np.concatenate(outs, axis=0)


if __name__ == "__main__":
    rng = np.random.default_rng(0)
    x = rng.standard_normal((B_FULL, H, W, C), dtype=np.float32)
    y = kernel(x)
    ref = x.reshape(B_FULL, BINS, BIN, BINS, BIN, C).max(axis=(2, 4))
    err = np.abs(y - ref).max()
    print("max abs err:", err)


# revision 17
# speedup vs baseline: 1.1892x; 1.1892x over previous
"""Adaptive max-pool [32,224,224,128] -> [32,7,7,128] on 8 TRN2 NeuronCores.

Data-parallel over batch: each core pools its own [4,224,224,128] slab.
224 = 7 bins x 32, so this is a 32x32/stride-32 max pool.

Per-core plan (memory-bound, ~103 MB HBM read per core). All compute on DVE
(vector engine) so cross-engine sync is minimal:

  Stage A: tiles [128 rows=(b,h), wchunk*32*128c] with 32KB-contiguous
           DMA descriptors; reduce_max over w (innermost via permuted AP)
           -> S1[row, j, c]   (j = w-bin).
  Stage B: DVE 32x32 block transpose of S1 [128, 896]: free block B=(j,cb),
           S2[32a+i, 32B+k] = S1[32a+k, 32B+i]  (a = h-bin in tile, k = row
           in bin, cb = c//32, i = c%32).
  Stage C: reduce_max over k (innermost 32) -> S3[32a+i, rt*32 + (j*4+cb)].
  Stage D: one more block transpose puts (j,cb) into partitions and (rt,i)
           into the free axis -> S4[32a + j*4+cb, rt*32+i]; 4 strided DMAs
           (one per a) scatter to out[(4rt+a), j, 32cb+i].
"""

import sys

if "/opt/trn_rl_repo" not in sys.path:
    sys.path.insert(0, "/opt/trn_rl_repo")

import numpy as np

import concourse.bacc as bacc
import concourse.bass as bass
import concourse.tile as tile
from concourse import mybir
from concourse.bass_utils import run_bass_kernel_spmd

N_CORES = 8
B_FULL, H, W, C = 32, 224, 224, 128
B = B_FULL // N_CORES  # 4 per core
BINS = 7
BIN = 32
ROWS = B * H  # 896 (b,h) rows per core
P = 128
N_ROW_TILES = ROWS // P  # 7
GB_PER_TILE = P // BIN  # 4 h-bins inside each 128-row tile

# w chunking for stage A: mostly 2-bin chunks (32KB/partition, 4MB DMAs —
# best HBM descriptor efficiency), with a small first chunk so DVE starts
# early and small last-row-tile chunks so the post-stream DVE tail is short.
CHUNKS_HEAD = [(0, 1), (1, 2), (3, 2), (5, 2)]
CHUNKS_MAIN = [(0, 2), (2, 2), (4, 2), (6, 1)]
CHUNKS_TAIL = [(j, 1) for j in range(BINS)]


def build_kernel() -> bass.Bass:
    nc = bacc.Bacc()
    f32 = mybir.dt.float32

    inp = nc.declare_dram_parameter("inputs", [B, H, W, C], f32, isOutput=False)
    out = nc.declare_dram_parameter("out", [B, BINS, BINS, C], f32, isOutput=True)

    in_flat = inp[:, :, :, :].rearrange("b h w c -> (b h) (w c)")  # [896, 28672]
    out_1d = out[:, :, :, :].rearrange("b i j c -> (b i j c)")  # [25088]

    with tile.TileContext(nc) as tc:
        with (
            tc.tile_pool(name="in_pool", bufs=5) as in_pool,
            tc.tile_pool(name="s1_pool", bufs=2) as s1_pool,
            tc.tile_pool(name="s2_pool", bufs=2) as s2_pool,
            tc.tile_pool(name="singles", bufs=1) as singles,
        ):
            s3 = singles.tile([P, N_ROW_TILES, BIN], f32)  # [p', rt, jcb(pad 32)]
            nc.vector.memset(s3, 0.0)
            s4 = singles.tile([P, N_ROW_TILES, BIN], f32)  # [q, rt, i]

            n_dma = 0
            for rt in range(N_ROW_TILES):
                s1 = s1_pool.tile([P, BINS, C], f32, tag="s1")  # [row, j, c]
                r0 = rt * P
                # ---- Stage A: load + contiguous pairwise max tree over w ----
                if rt == 0:
                    chunks = CHUNKS_HEAD
                elif rt == N_ROW_TILES - 1:
                    chunks = CHUNKS_TAIL
                else:
                    chunks = CHUNKS_MAIN
                for j0, nj in chunks:
                    t = in_pool.tile([P, nj, BIN, C], f32, tag="in")
                    # alternate the two HWDGE rings (SP / ACT) so DMA fixed
                    # costs overlap
                    eng = nc.sync if n_dma % 2 == 0 else nc.scalar
                    n_dma += 1
                    eng.dma_start(
                        out=t,
                        in_=in_flat[r0 : r0 + P, j0 * BIN * C : (j0 + nj) * BIN * C],
                    )
                    # strided reduce runs at ~0.62 elem/cycle on DVE; a
                    # contiguous in-place halving tree runs at ~1/cycle
                    w = BIN
                    while w > 2:
                        h = w // 2
                        nc.vector.tensor_max(
                            t[:, :, 0:h, :], t[:, :, 0:h, :], t[:, :, h:w, :]
                        )
                        w = h
                    nc.vector.tensor_max(
                        s1[:, j0 : j0 + nj, :], t[:, :, 0, :], t[:, :, 1, :]
                    )
                # ---- Stage B: 32x32 block transpose ----
                s2 = s2_pool.tile([P, BINS * C], f32, tag="s2")
                nc.vector.transpose(s2, s1.rearrange("p j c -> p (j c)"))
                # ---- Stage C: reduce over rows-in-bin (k) ----
                nc.vector.reduce_max(
                    out=s3[:, rt, 0 : BINS * GB_PER_TILE],
                    in_=s2.rearrange("p (bb k) -> p bb k", k=BIN),
                    axis=mybir.AxisListType.X,
                )
                # ---- Stage D (per rt): block transpose of this rt column ----
                nc.vector.transpose(s4[:, rt, :], s3[:, rt, :])
                # ---- output DMAs for this rt (one per h-bin slot a) ----
                # out flat offset = (4*rt + a)*896 + j*128 + 32*cb + i
                for a in range(GB_PER_TILE):
                    dst = bass.AP(
                        tensor=out_1d.tensor,
                        offset=(GB_PER_TILE * rt + a) * BINS * C,
                        ap=[
                            [C, BINS],  # j
                            [BIN, GB_PER_TILE],  # cb
                            [1, BIN],  # i
                        ],
                    )
                    # SWDGE keeps output traffic off the load HWDGE rings
                    nc.gpsimd.dma_start(
                        out=dst,
                        in_=s4[a * BIN : a * BIN + BINS * GB_PER_TILE, rt, :],
                    )

    nc.compile()
    return nc


_NC_CACHE = None


def kernel(inputs: np.ndarray) -> np.ndarray:
    global _NC_CACHE
    inputs = np.ascontiguousarray(inputs, dtype=np.float32)
    assert inputs.shape == (B_FULL, H, W, C)

    if _NC_CACHE is None:
        _NC_CACHE = build_kernel()
    nc = _NC_CACHE

    shards = np.split(inputs, N_CORES, axis=0)
    in_maps = [{"inputs": s} for s in shards]
    res = run_bass_kernel_spmd(nc, in_maps, core_ids=list(range(N_CORES)))
    outs = [res.results[i]["out"] for i in range(N_CORES)]
    return np.concatenate(outs, axis=0)


if __name__ == "__main__":
    rng = np.random.default_rng(0)
    x = rng.standard_normal((B_FULL, H, W, C), dtype=np.float32)
    y = kernel(x)
    ref = x.reshape(B_FULL, BINS, BIN, BINS, BIN, C).max(axis=(2, 4))
    err = np.abs(y - ref).max()
    print("max abs err:", err)
